# revision 58
# baseline (speedup 1.0000x reference)
"""KBertGATEnricher Trainium2 kernel.

Sharding: data-parallel over batch (8 batches -> 8 cores) for embedding+GAT,
then a split AllGather of the concatenated head features (heads 0-1 fire
early so the output GEMM starts sooner), then vocab-column-parallel output
Linear + global log_softmax (grouped AllReduce of per-token sum-exp,
overlapped with later GEMM work; final subtract split across the vector and
scalar engines).

Self-contained: hardcodes all shapes; only imports the system-installed
concourse runtime.
"""

import os
import sys

sys.path.insert(0, "/opt/trn_rl_repo")

import numpy as np

from concourse import bass, bacc, mybir, tile
from concourse.bass_utils import run_bass_kernel_spmd

F32 = mybir.dt.float32
F16 = mybir.dt.float16

B, N, D, H, F, V = 8, 256, 768, 4, 128, 30522
NCORES = 8
VS = 3816          # per-core vocab columns (8*3816 = 30528, 6 pad cols)
VPAD = VS * NCORES
NPADC = float(VPAD - V)  # padded weight columns, each contributes exp(0)=1
LN_EPS = 1e-12
ALPHA = 0.01       # leaky relu slope
MASK_NEG = -5000.0  # pre-leaky masked logit; leaky -> ~-50 -> exp ~ 0
NKT = D // 128     # 6 hidden k-tiles for the GAT matmuls
NM = (B * N) // 128  # 16 token m-tiles
CW = 512           # phase-B moving width (PSUM bank limit for f32 out)
CHUNKS = [(c0, min(CW, VS - c0)) for c0 in range(0, VS, CW)]  # 8 chunks
GB = [0, 6, 12, 14, 16]  # AllReduce groups; small last groups keep the
                         # tail (last AR + its finals) short
NG = len(GB) - 1
QW = 954           # final stage width (VS = 4*954)
QCHUNKS = [(c0, min(QW, VS - c0)) for c0 in range(0, VS, QW)]
DW = 1908          # dum half width
DCHUNKS = [(0, DW), (DW, DW)]
DREADY = [3, 7]    # GEMM chunk after which each dum half's range is done
LN2 = 0.6931471805599453
# ln(1+t) ~ sum a_k t^k on [0,1), max err 1.2e-5
LNC = [0.9994349429297625, -0.49134746165823384, 0.2878246937290064,
       -0.13413330582888625, 0.03137662229933151]

AX = mybir.AxisListType
AF = mybir.ActivationFunctionType
OP = mybir.AluOpType

_NC_CACHE = {}


def _build(with_ln_b: bool, with_out_b: bool):
    """Build the SPMD Bass program (identical on all 8 cores)."""
    nc = bacc.Bacc(
        "TRN2",
        target_bir_lowering=False,
        debug=False,
        enable_asserts=False,
        num_devices=NCORES,
    )

    # ---- per-core I/O --------------------------------------------------
    xpre = nc.dram_tensor("xpre", [N, D], F32, kind="ExternalInput").ap()
    mneg = nc.dram_tensor("mneg", [N, N], F16, kind="ExternalInput").ap()
    wmat = nc.dram_tensor("wmat", [D, H * F], F16, kind="ExternalInput").ap()
    wsum = nc.dram_tensor("wsum", [D, 2 * H], F16, kind="ExternalInput").ap()
    wst = nc.dram_tensor("wst", [4, 128, VS], F16, kind="ExternalInput").ap()
    if with_ln_b:
        browm = nc.dram_tensor("browm", [1, H * F], F16, kind="ExternalInput").ap()
        brows = nc.dram_tensor("brows", [1, 2 * H], F16, kind="ExternalInput").ap()
    if with_out_b:
        bvoc = nc.dram_tensor("bvoc", [1, VS], F16, kind="ExternalInput").ap()
    # f16 output staging: host converts to f32 (adds <0.008 abs err, budget
    # is ~0.46); halves the 31MB/core output DMA and doubles DVE throughput
    # on the final subtract.
    out = nc.dram_tensor("out", [B * N, VS], F16, kind="ExternalOutput").ap()

    rg = [list(range(NCORES))]

    with tile.TileContext(nc) as tc:
        # ---- persistent SBUF ------------------------------------------
        with (
            tc.tile_pool(name="wpool", bufs=1) as wpool,
            tc.tile_pool(name="catf_pool", bufs=1) as catf_pool,
            tc.tile_pool(name="dram", bufs=1, space="DRAM") as dram,
        ):
            w_sb = [wpool.tile([128, VS], F16, tag=f"w{kt}", name=f"w{kt}") for kt in range(4)]
            catf = [
                catf_pool.tile([128, B * N], F16, tag=f"catf{kt}", name=f"catf{kt}") for kt in range(4)
            ]
            if with_out_b:
                bvoc_sb = wpool.tile([1, VS], F16, tag="bvoc")
                ones1v = wpool.tile([1, 128], F16, tag="ones1v")
                nc.vector.memset(ones1v[:], 1.0)

            cc_in = dram.tile([H * F, N], F16, name="cc_in")
            cc_out = dram.tile(
                [NCORES, H * F, N], F16, addr_space="Shared", name="cc_out"
            )
            sum_in = [
                dram.tile([128, GB[g + 1] - GB[g]], F32, name=f"sum_in{g}")
                for g in range(NG)
            ]
            sum_out = [
                dram.tile(
                    [128, GB[g + 1] - GB[g]], F32, addr_space="Shared", name=f"sum_out{g}"
                )
                for g in range(NG)
            ]

            # ==== phase A: embedding LN + GAT (own batch) ==============
            with (
                tc.tile_pool(name="pa", bufs=1) as pa,
                tc.tile_pool(name="pa_tmp", bufs=2) as pa_tmp,
                tc.tile_pool(name="ps_a", bufs=1, space="PSUM") as ps_a,
            ):
                # input DMAs in latency order: LN input first, the big
                # vocab weights (not needed until phase B) last.
                xp_sb = [pa.tile([128, D], F32, tag=f"xp{m}", name=f"xp{m}") for m in range(2)]
                for m in range(2):
                    nc.sync.dma_start(
                        out=xp_sb[m][:], in_=xpre[m * 128 : (m + 1) * 128, :]
                    )
                mneg_sb = [pa.tile([128, N], F16, tag=f"mneg{j}", name=f"mneg{j}") for j in range(2)]
                for jt in range(2):
                    nc.sync.dma_start(
                        out=mneg_sb[jt][:], in_=mneg[jt * 128 : (jt + 1) * 128, :]
                    )
                wmat_sb = [
                    pa.tile([128, H * F], F16, tag=f"wmat{kt}", name=f"wmat{kt}")
                    for kt in range(NKT)
                ]
                wsum_sb = [
                    pa.tile([128, 2 * H], F16, tag=f"wsum{kt}", name=f"wsum{kt}")
                    for kt in range(NKT)
                ]
                for kt in range(NKT):
                    nc.sync.dma_start(
                        out=wmat_sb[kt][:], in_=wmat[kt * 128 : (kt + 1) * 128, :]
                    )
                    nc.sync.dma_start(
                        out=wsum_sb[kt][:], in_=wsum[kt * 128 : (kt + 1) * 128, :]
                    )
                if with_ln_b:
                    browm_sb = pa.tile([1, H * F], F16, tag="browm")
                    nc.sync.dma_start(out=browm_sb[:], in_=browm[:, :])
                    brows_sb = pa.tile([1, 2 * H], F16, tag="brows")
                    nc.sync.dma_start(out=brows_sb[:], in_=brows[:, :])
                for kt in range(4):
                    nc.sync.dma_start(out=w_sb[kt][:], in_=wst[kt, :, :])
                if with_out_b:
                    nc.sync.dma_start(out=bvoc_sb[:], in_=bvoc[:, :])

                idw = pa.tile([128, 128], F16, tag="idw")
                bass_masks_identity(nc, idw[:])
                ones1 = pa.tile([1, 128], F16, tag="ones1")
                nc.vector.memset(ones1[:], 1.0)
                eps_sb = pa.tile([128, 1], F32, tag="eps_sb")
                nc.vector.memset(eps_sb[:], LN_EPS)

                # ---- LayerNorm (tokens on partitions) -----------------
                # mean via vector reduce in parallel with sum-of-squares via
                # scalar Square+accum; then var = E[x^2]-mu^2 on tiny tiles.
                xn_sb = [pa.tile([128, D], F16, tag=f"xn{m}", name=f"xn{m}") for m in range(2)]
                for m in range(2):
                    xp = xp_sb[m]
                    xsum = pa_tmp.tile([128, 1], F32, tag="xsum")
                    nc.vector.tensor_reduce(
                        out=xsum[:], in_=xp[:], axis=AX.X, op=OP.add
                    )
                    sq = pa_tmp.tile([128, D], F32, tag="sq")
                    ssum = pa_tmp.tile([128, 1], F32, tag="ssum")
                    nc.scalar.activation(
                        sq[:], xp[:], AF.Square, accum_out=ssum[:, 0:1]
                    )
                    mu = pa_tmp.tile([128, 1], F32, tag="mu")
                    nc.vector.tensor_scalar_mul(mu[:], xsum[:], 1.0 / D)
                    ex2 = pa_tmp.tile([128, 1], F32, tag="ex2")
                    nc.vector.tensor_scalar_mul(ex2[:], ssum[:], 1.0 / D)
                    mu2 = pa_tmp.tile([128, 1], F32, tag="mu2")
                    nc.vector.tensor_scalar_mul(mu2[:], mu[:], mu[:, 0:1])
                    var = pa_tmp.tile([128, 1], F32, tag="var")
                    nc.vector.scalar_tensor_tensor(
                        var[:], ex2[:], 0.0, mu2[:], OP.add, OP.subtract
                    )
                    sd = pa_tmp.tile([128, 1], F32, tag="sd")
                    nc.scalar.activation(
                        sd[:], var[:], AF.Sqrt, bias=eps_sb[:, 0:1]
                    )
                    rstd = pa_tmp.tile([128, 1], F32, tag="rstd")
                    nc.vector.reciprocal(rstd[:], sd[:])
                    nc.vector.tensor_scalar(
                        xn_sb[m][:], xp[:], mu[:, 0:1], rstd[:, 0:1],
                        OP.subtract, OP.mult,
                    )

                # ---- transpose xn -> xT[kt] [128 hid, 256 tok] --------
                xt_sb = [pa.tile([128, N], F16, tag=f"xt{kt}", name=f"xt{kt}") for kt in range(NKT)]
                for kt in range(NKT):
                    for m in range(2):
                        ptr = ps_a.tile([128, 128], F16, tag="ptr", bufs=2)
                        nc.tensor.transpose(
                            ptr[:], xn_sb[m][:, kt * 128 : (kt + 1) * 128], idw[:]
                        )
                        nc.vector.tensor_scalar_mul(
                            xt_sb[kt][:, m * 128 : (m + 1) * 128], ptr[:], 1.0
                        )

                # ---- all-heads Wh GEMM + s1/s2 contractions -----------
                wh_all = [pa.tile([128, H * F], F16, tag=f"whall{m}", name=f"whall{m}") for m in range(2)]
                s12m = [pa.tile([128, 2 * H], F32, tag=f"s12m{m}", name=f"s12m{m}") for m in range(2)]
                s1r = [pa.tile([1, N], F16, tag=f"s1r{h}", name=f"s1r{h}") for h in range(H)]
                for m in range(2):
                    pwh = ps_a.tile([128, H * F], F32, tag="pwh", bufs=2)
                    for kt in range(NKT):
                        nc.tensor.matmul(
                            pwh[:],
                            xt_sb[kt][:, m * 128 : (m + 1) * 128],
                            wmat_sb[kt][:],
                            start=(kt == 0),
                            stop=(kt == NKT - 1) and not with_ln_b,
                        )
                    if with_ln_b:
                        nc.tensor.matmul(
                            pwh[:], ones1[:], browm_sb[:], start=False, stop=True
                        )
                    nc.vector.tensor_scalar_mul(wh_all[m][:], pwh[:], 1.0)

                    pws = ps_a.tile([128, 2 * H], F32, tag="pws", bufs=1)
                    for kt in range(NKT):
                        nc.tensor.matmul(
                            pws[:],
                            xt_sb[kt][:, m * 128 : (m + 1) * 128],
                            wsum_sb[kt][:],
                            start=(kt == 0),
                            stop=(kt == NKT - 1) and not with_ln_b,
                        )
                    if with_ln_b:
                        nc.tensor.matmul(
                            pws[:], ones1[:], brows_sb[:], start=False, stop=True
                        )
                    nc.vector.tensor_scalar_mul(s12m[m][:], pws[:], 1.0)
                    # per-head s1 column [128,1] -> row [1,128] via transpose
                    s12h = pa_tmp.tile([128, H], F16, tag="s12h")
                    nc.vector.tensor_scalar_mul(s12h[:], pws[:, 0:H], 1.0)
                    for h in range(H):
                        ps1h = ps_a.tile([1, 128], F16, tag="ps1h", bufs=1)
                        nc.tensor.transpose(ps1h[:], s12h[:, h : h + 1], idw[:])
                        nc.vector.tensor_scalar_mul(s1r[h][:, m * 128 : (m + 1) * 128], ps1h[:], 1.0)

                # ---- per-head attention + aggregation -----------------
                att = [
                    [pa.tile([128, N], F16, tag=f"att{h}_{m}", name=f"att{h}_{m}") for m in range(2)]
                    for h in range(H)
                ]
                cat_sb = [pa.tile([128, N], F16, tag=f"cat{h}", name=f"cat{h}") for h in range(H)]

                for h in range(H):
                    # attention scores + column softmax (over i = free dim)
                    for jt in range(2):
                        # psum = broadcast(s1) + (-5000)*mask
                        pet = ps_a.tile([128, N], F32, tag="pet", bufs=2)
                        nc.tensor.matmul(
                            pet[:], ones1[:], s1r[h][:], start=True, stop=False
                        )
                        nc.tensor.matmul(
                            pet[:], idw[:], mneg_sb[jt][:], start=False, stop=True
                        )
                        et = pa_tmp.tile([128, N], F32, tag="et")
                        nc.vector.tensor_scalar_add(
                            et[:], pet[:], s12m[jt][:, H + h : H + h + 1]
                        )
                        lr = pa_tmp.tile([128, N], F32, tag="lr")
                        nc.vector.scalar_tensor_tensor(
                            lr[:], et[:], ALPHA, et[:], OP.mult, OP.max
                        )
                        # softmax without max-subtraction (f32 exp; max ~e19)
                        ex = pa_tmp.tile([128, N], F32, tag="ex")
                        asum = pa_tmp.tile([128, 1], F32, tag="asum")
                        nc.scalar.activation(
                            ex[:], lr[:], AF.Exp, accum_out=asum[:, 0:1]
                        )
                        rec = pa_tmp.tile([128, 1], F32, tag="rec")
                        nc.vector.reciprocal(rec[:], asum[:])
                        nc.vector.tensor_scalar_mul(
                            att[h][jt][:], ex[:], rec[:, 0:1]
                        )

                    # hp^T = Wh^T @ att^T, then elu -> catT rows of head h
                    php = ps_a.tile([128, N], F32, tag="pwh", bufs=2)
                    for jt in range(2):
                        nc.tensor.matmul(
                            php[:],
                            wh_all[jt][:, h * F : (h + 1) * F],
                            att[h][jt][:],
                            start=(jt == 0),
                            stop=(jt == 1),
                        )
                    e0h = pa_tmp.tile([128, N], F16, tag="e0h")
                    nc.scalar.activation(e0h[:], php[:], AF.Exp)
                    tmh = pa_tmp.tile([128, N], F16, tag="tmh")
                    nc.vector.tensor_scalar(
                        tmh[:], e0h[:], 1.0, -1.0, OP.min, OP.add
                    )
                    nc.vector.scalar_tensor_tensor(
                        cat_sb[h][:], php[:], 0.0, tmh[:], OP.max, OP.add
                    )
                    nc.sync.dma_start(
                        out=cc_in[h * F : (h + 1) * F, :],
                        in_=cat_sb[h][:],
                    )
                nc.gpsimd.collective_compute(
                    "AllGather",
                    OP.bypass,
                    replica_groups=rg,
                    ins=[cc_in.opt()],
                    outs=[cc_out.opt()],
                )
                for kt in range(4):
                    nc.sync.dma_start(
                        out=catf[kt][:].rearrange("p (r n) -> p r n", r=NCORES),
                        in_=cc_out[:, kt * F : (kt + 1) * F, :].rearrange(
                            "r p n -> p r n"
                        ),
                    )

            # ==== vocab-parallel output linear + log_softmax ===========
            with (
                tc.tile_pool(name="vp_pool", bufs=1) as vp_pool,
                tc.tile_pool(name="big_tmp", bufs=4) as big_tmp,
                tc.tile_pool(name="stat", bufs=1) as stat,
                tc.tile_pool(name="stage_pool", bufs=4) as stage_pool,
                tc.tile_pool(name="ps_z", bufs=8, space="PSUM") as ps_z,
            ):
                # q[m] holds elu(z) for m-tile m (f16)
                qt = [
                    vp_pool.tile([128, VS], F16, tag=f"q{m}", name=f"q{m}") for m in range(NM)
                ]
                lsq = stat.tile([128, NM * 2], F32, tag="lsq")
                gsum = stat.tile([128, NM], F32, tag="gsum")
                logl = stat.tile([128, NM], F32, tag="logl")
                nlogl = stat.tile([128, NM], F32, tag="nlogl")

                pend_dum = []

                def emit_dum(m, qi):
                    qc0, qcw = DCHUNKS[qi]
                    dum = big_tmp.tile([128, DW], F16, tag="dum", bufs=2)
                    nc.scalar.activation(
                        dum[:, 0:qcw],
                        qt[m][:, qc0 : qc0 + qcw],
                        AF.Exp,
                        accum_out=lsq[:, m * 2 + qi : m * 2 + qi + 1],
                    )

                def do_mtile(m):
                    """GEMM + elu chunks; dum halves interleaved.

                    The second dum half is deferred into the NEXT m-tile's
                    chunk stream (after its first exp1) so the scalar queue
                    never delays PSUM recycling at an m-tile boundary.
                    """
                    for ci, (c0, cw) in enumerate(CHUNKS):
                        zp = ps_z.tile([128, CW], F32, tag="z")
                        for kt in range(4):
                            nc.tensor.matmul(
                                zp[:, 0:cw],
                                catf[kt][:, m * 128 : (m + 1) * 128],
                                w_sb[kt][:, c0 : c0 + cw],
                                start=(kt == 0),
                                stop=(kt == 3) and not with_out_b,
                            )
                        if with_out_b:
                            nc.tensor.matmul(
                                zp[:, 0:cw],
                                ones1v[:],
                                bvoc_sb[:, c0 : c0 + cw],
                                start=False,
                                stop=True,
                            )
                        # e0 = exp(z) (f16; overflow->inf is clamped by min)
                        e0 = big_tmp.tile([128, CW], F16, tag="e0")
                        nc.scalar.activation(e0[:, 0:cw], zp[:, 0:cw], AF.Exp)
                        # tmin1 = min(e0,1) - 1
                        tm = big_tmp.tile([128, CW], F16, tag="tm")
                        nc.vector.tensor_scalar(
                            tm[:, 0:cw], e0[:, 0:cw], 1.0, -1.0, OP.min, OP.add
                        )
                        # q = max(z,0) + tmin1  == elu(z)
                        nc.vector.scalar_tensor_tensor(
                            qt[m][:, c0 : c0 + cw],
                            zp[:, 0:cw],
                            0.0,
                            tm[:, 0:cw],
                            OP.max,
                            OP.add,
                        )
                        # spread the exp-sum work across the chunk stream
                        if ci == 1 and pend_dum:
                            emit_dum(*pend_dum.pop(0))
                        elif ci == 4:
                            emit_dum(m, 0)
                    pend_dum.append((m, 1))

                def start_group_ar(g):
                    """Reduce group sums and fire its AllReduce.

                    The tiny sum DMAs ride the gpsimd DMA queue so they are
                    not stuck behind megabytes of output DMA on sync's queue.
                    """
                    g0, g1 = GB[g], GB[g + 1]
                    gm = g1 - g0
                    while pend_dum:
                        emit_dum(*pend_dum.pop(0))
                    ls = stat.tile([128, 8], F32, tag="ls", name=f"ls{g}")
                    nc.vector.tensor_reduce(
                        out=ls[:, 0:gm],
                        in_=lsq[:, g0 * 2 : g1 * 2].rearrange(
                            "p (m q) -> p m q", q=2
                        ),
                        axis=AX.X,
                        op=OP.add,
                    )
                    nc.gpsimd.dma_start(out=sum_in[g][:], in_=ls[:, 0:gm])
                    nc.gpsimd.collective_compute(
                        "AllReduce",
                        OP.add,
                        replica_groups=rg,
                        ins=[sum_in[g].opt()],
                        outs=[sum_out[g].opt()],
                    )
                def compute_logl(g):
                    """ln of the reduced sums, on the vector engine
                    (exponent bit-extract + degree-5 polynomial), so the
                    scalar activation table never leaves the exp set."""
                    g0, g1 = GB[g], GB[g + 1]
                    gm = g1 - g0
                    nc.gpsimd.dma_start(out=gsum[:, g0:g1], in_=sum_out[g][:])
                    # pad columns contribute exp(0)=1 each: subtract first
                    gs = gsum[:, g0:g1]
                    s6 = stat.tile([128, 8], F32, tag="s6", name=f"s6{g}")
                    nc.vector.tensor_scalar_add(s6[:, 0:gm], gs, -NPADC)
                    ui = s6[:, 0:gm].bitcast(mybir.dt.uint32)
                    ei = stat.tile([128, 8], mybir.dt.uint32, tag="ei", name=f"ei{g}")
                    nc.vector.tensor_scalar(
                        ei[:, 0:gm], ui, 23, None, OP.logical_shift_right
                    )
                    ef = stat.tile([128, 8], F32, tag="ef", name=f"ef{g}")
                    nc.vector.tensor_scalar_mul(ef[:, 0:gm], ei[:, 0:gm], 1.0)
                    mi = stat.tile([128, 8], mybir.dt.uint32, tag="mi", name=f"mi{g}")
                    nc.vector.tensor_scalar(
                        mi[:, 0:gm], ui, 0x007FFFFF, 0x3F800000,
                        OP.bitwise_and, OP.bitwise_or,
                    )
                    tv = stat.tile([128, 8], F32, tag="tv", name=f"tv{g}")
                    nc.vector.tensor_scalar_sub(
                        tv[:, 0:gm], mi[:, 0:gm].bitcast(F32), 1.0
                    )
                    pz = stat.tile([128, 8], F32, tag="pz", name=f"pz{g}")
                    nc.vector.tensor_scalar_mul(pz[:, 0:gm], tv[:, 0:gm], LNC[4])
                    for a in (LNC[3], LNC[2], LNC[1], LNC[0]):
                        nc.vector.scalar_tensor_tensor(
                            pz[:, 0:gm], pz[:, 0:gm], a, tv[:, 0:gm],
                            OP.add, OP.mult,
                        )
                    # exponent field is still biased by 127: fold into poly
                    nc.vector.tensor_scalar_add(
                        pz[:, 0:gm], pz[:, 0:gm], -127.0 * LN2
                    )
                    nc.vector.scalar_tensor_tensor(
                        logl[:, g0:g1], ef[:, 0:gm], LN2, pz[:, 0:gm],
                        OP.mult, OP.add,
                    )
                    nc.vector.tensor_scalar_mul(nlogl[:, g0:g1], logl[:, g0:g1], -1.0)

                def emit_final_mtile(m, last_group=False):
                    """out = q - log(L), staged f16: 3/4 vector, 1/4 scalar."""
                    for oi, (c0, cw) in enumerate(QCHUNKS):
                        stg = stage_pool.tile([128, QW], F16, tag="stg")
                        # tail groups: both engines are idle, so split the
                        # serial final stages between scalar and vector
                        on_scalar = last_group and oi % 2 == 1
                        if not on_scalar:
                            nc.vector.tensor_scalar_sub(
                                stg[:, 0:cw],
                                qt[m][:, c0 : c0 + cw],
                                logl[:, m : m + 1],
                            )
                        else:
                            nc.scalar.activation(
                                stg[:, 0:cw],
                                qt[m][:, c0 : c0 + cw],
                                AF.Identity,
                                bias=nlogl[:, m : m + 1],
                            )
                        nc.sync.dma_start(
                            out=out[m * 128 : (m + 1) * 128, c0 : c0 + cw],
                            in_=stg[:, 0:cw],
                        )

                # staggered schedule: group g's AllReduce fires right after
                # its compute; its ln chain + finals are emitted a group
                # later (AR long done -> no queue stall), with the final
                # stages interleaved between m-tiles so they never form a
                # block that delays the next group's elementwise work.
                for g in range(NG):
                    for m in range(GB[g], GB[g + 1]):
                        do_mtile(m)
                    start_group_ar(g)
                    if g >= 1:
                        compute_logl(g - 1)
                        for m in range(GB[g - 1], GB[g]):
                            emit_final_mtile(m)
                compute_logl(NG - 1)
                for m in range(GB[NG - 1], GB[NG]):
                    emit_final_mtile(m, last_group=True)

    nc.compile()
    return nc


def bass_masks_identity(nc, ident_ap):
    from concourse import masks

    masks.make_identity(nc, ident_ap)


def _host_prep(inputs):
    """Per-core input maps from full inputs (numpy only)."""
    tok = np.asarray(inputs["token_ids"])
    typ = np.asarray(inputs["type_ids"])
    syn = np.asarray(inputs["synset_ids"])
    hw = np.asarray(inputs["highway"]).astype(bool)
    tok_emb = np.asarray(inputs["tok_emb"], dtype=np.float32)
    type_emb = np.asarray(inputs["type_emb"], dtype=np.float32)
    pos_emb = np.asarray(inputs["pos_emb"], dtype=np.float32)
    ln_g = np.asarray(inputs["ln_g"], dtype=np.float32)
    ln_b = np.asarray(inputs["ln_b"], dtype=np.float32)
    W = np.asarray(inputs["W"], dtype=np.float32)
    a = np.asarray(inputs["a"], dtype=np.float32)
    out_W = np.asarray(inputs["out_W"], dtype=np.float32)
    out_b = np.asarray(inputs["out_b"], dtype=np.float32)

    # embeddings (host gather + add, f32 like the reference)
    x_pre = tok_emb[tok] + type_emb[typ] + pos_emb[:N][None]  # (B,N,D)

    # graph mask (host index logic), transposed to [j, i]
    vis = syn[:, :, None] == syn[:, None, :]
    s1m = (typ == 1) & hw
    s3m = (typ == 3) & hw
    d1 = np.isin(typ, [0, 2, 5]) & hw
    d3 = np.isin(typ, [6, 4, 0]) & hw
    vis = vis | (s1m[:, :, None] & d1[:, None, :]) | (s3m[:, :, None] & d3[:, None, :])
    mask = vis & (tok != 0)[:, None, :]  # (B,N,N) over [i,j]
    # -5000 where masked-out, 0 where visible; [j, i] layout
    mneg = np.where(mask.transpose(0, 2, 1), 0.0, MASK_NEG).astype(np.float16)

    # GAT weights: fold ln_g; separate Wh matrix and a1/a2 contractions
    Wg = W * ln_g[None, :, None]  # (H,D,F)
    a1, a2 = a[:, :F], a[:, F:]
    c1 = np.einsum("hdf,hf->hd", Wg, a1)  # (H,D)
    c2 = np.einsum("hdf,hf->hd", Wg, a2)
    wmat = Wg.transpose(1, 0, 2).reshape(D, H * F).astype(np.float16)
    wsum = np.concatenate([c1.T, c2.T], axis=1).astype(np.float16)  # (D, 2H)

    with_ln_b = bool(np.any(ln_b != 0.0))
    browm = brows = None
    if with_ln_b:
        b1 = np.einsum("hdf,hf->hd", W, a1)  # (H,D)
        b2 = np.einsum("hdf,hf->hd", W, a2)
        browm = np.einsum("d,hdf->hf", ln_b, W).reshape(1, H * F).astype(np.float16)
        brows = np.concatenate([b1 @ ln_b, b2 @ ln_b]).reshape(1, 2 * H).astype(
            np.float16
        )

    # vocab shards of out_W^T (padded to 30528)
    wpad = np.zeros((VPAD, H * F), dtype=np.float32)
    wpad[:V] = out_W
    with_out_b = bool(np.any(out_b != 0.0))
    bpad = np.zeros((VPAD,), dtype=np.float32)
    bpad[:V] = out_b

    in_maps = []
    for c in range(NCORES):
        wc = wpad[c * VS : (c + 1) * VS].T.astype(np.float16)  # (512, VS)
        m = {
            "xpre": np.ascontiguousarray(x_pre[c]),
            "mneg": np.ascontiguousarray(mneg[c]),
            "wmat": wmat,
            "wsum": wsum,
            "wst": np.ascontiguousarray(wc.reshape(4, 128, VS)),
        }
        if with_ln_b:
            m["browm"] = browm
            m["brows"] = brows
        if with_out_b:
            m["bvoc"] = np.ascontiguousarray(
                bpad[c * VS : (c + 1) * VS].reshape(1, VS).astype(np.float16)
            )
        in_maps.append(m)
    return in_maps, with_ln_b, with_out_b


def kernel(**inputs) -> np.ndarray:
    in_maps, with_ln_b, with_out_b = _host_prep(inputs)

    key = (with_ln_b, with_out_b)
    if key not in _NC_CACHE:
        _NC_CACHE[key] = _build(with_ln_b, with_out_b)
    nc = _NC_CACHE[key]

    trace = bool(int(os.environ.get("KBERT_TRACE", "0")))
    res = run_bass_kernel_spmd(
        nc, in_maps, core_ids=list(range(NCORES)), trace=trace
    )
    if trace and res.exec_time_ns is not None:
        print(f"HW exec time: {res.exec_time_ns} ns")
        if res.instructions_and_trace is not None:
            print(f"trace: {res.instructions_and_trace[1]}")

    full = np.empty((B * N, VPAD), dtype=np.float32)
    for c in range(NCORES):
        full[:, c * VS : (c + 1) * VS] = res.results[c]["out"]
    return np.ascontiguousarray(full[:, :V].reshape(B, N, V))


# revision 59
# speedup vs baseline: 1.0314x; 1.0314x over previous
"""KBertGATEnricher Trainium2 kernel.

Sharding: data-parallel over batch (8 batches -> 8 cores) for embedding+GAT,
then a split AllGather of the concatenated head features (heads 0-1 fire
early so the output GEMM starts sooner), then vocab-column-parallel output
Linear + global log_softmax (grouped AllReduce of per-token sum-exp,
overlapped with later GEMM work; final subtract split across the vector and
scalar engines).

Self-contained: hardcodes all shapes; only imports the system-installed
concourse runtime.
"""

import os
import sys

sys.path.insert(0, "/opt/trn_rl_repo")

import numpy as np

from concourse import bass, bacc, mybir, tile
from concourse.bass_utils import run_bass_kernel_spmd

F32 = mybir.dt.float32
F16 = mybir.dt.float16

B, N, D, H, F, V = 8, 256, 768, 4, 128, 30522
NCORES = 8
VS = 3816          # per-core vocab columns (8*3816 = 30528, 6 pad cols)
VPAD = VS * NCORES
NPADC = float(VPAD - V)  # padded weight columns, each contributes exp(0)=1
LN_EPS = 1e-12
ALPHA = 0.01       # leaky relu slope
MASK_NEG = -5000.0  # pre-leaky masked logit; leaky -> ~-50 -> exp ~ 0
NKT = D // 128     # 6 hidden k-tiles for the GAT matmuls
NM = (B * N) // 128  # 16 token m-tiles
CW = 512           # phase-B moving width (PSUM bank limit for f32 out)
CHUNKS = [(c0, min(CW, VS - c0)) for c0 in range(0, VS, CW)]  # 8 chunks
GB = [0, 6, 12, 14, 16]  # AllReduce groups; small last groups keep the
                         # tail (last AR + its finals) short
NG = len(GB) - 1
QW = 954           # final stage width (VS = 4*954)
QCHUNKS = [(c0, min(QW, VS - c0)) for c0 in range(0, VS, QW)]
DW = 1908          # dum half width
DCHUNKS = [(0, DW), (DW, DW)]
DREADY = [3, 7]    # GEMM chunk after which each dum half's range is done
LN2 = 0.6931471805599453
# ln(1+t) ~ sum a_k t^k on [0,1), max err 1.2e-5
LNC = [0.9994349429297625, -0.49134746165823384, 0.2878246937290064,
       -0.13413330582888625, 0.03137662229933151]

AX = mybir.AxisListType
AF = mybir.ActivationFunctionType
OP = mybir.AluOpType

_NC_CACHE = {}


def _build(with_ln_b: bool, with_out_b: bool):
    """Build the SPMD Bass program (identical on all 8 cores)."""
    nc = bacc.Bacc(
        "TRN2",
        target_bir_lowering=False,
        debug=False,
        enable_asserts=False,
        num_devices=NCORES,
    )

    # ---- per-core I/O --------------------------------------------------
    xpre = nc.dram_tensor("xpre", [N, D], F32, kind="ExternalInput").ap()
    mneg = nc.dram_tensor("mneg", [N, N], F16, kind="ExternalInput").ap()
    wmat = nc.dram_tensor("wmat", [D, H * F], F16, kind="ExternalInput").ap()
    wsum = nc.dram_tensor("wsum", [D, 2 * H], F16, kind="ExternalInput").ap()
    wst = nc.dram_tensor("wst", [4, 128, VS], F16, kind="ExternalInput").ap()
    if with_ln_b:
        browm = nc.dram_tensor("browm", [1, H * F], F16, kind="ExternalInput").ap()
        brows = nc.dram_tensor("brows", [1, 2 * H], F16, kind="ExternalInput").ap()
    if with_out_b:
        bvoc = nc.dram_tensor("bvoc", [1, VS], F16, kind="ExternalInput").ap()
    # f16 output staging: host converts to f32 (adds <0.008 abs err, budget
    # is ~0.46); halves the 31MB/core output DMA and doubles DVE throughput
    # on the final subtract.
    out = nc.dram_tensor("out", [B * N, VS], F16, kind="ExternalOutput").ap()

    rg = [list(range(NCORES))]

    with tile.TileContext(nc) as tc:
        # ---- persistent SBUF ------------------------------------------
        with (
            tc.tile_pool(name="wpool", bufs=1) as wpool,
            tc.tile_pool(name="catf_pool", bufs=1) as catf_pool,
            tc.tile_pool(name="dram", bufs=1, space="DRAM") as dram,
        ):
            w_sb = [wpool.tile([128, VS], F16, tag=f"w{kt}", name=f"w{kt}") for kt in range(4)]
            catf = [
                catf_pool.tile([128, B * N], F16, tag=f"catf{kt}", name=f"catf{kt}") for kt in range(4)
            ]
            if with_out_b:
                bvoc_sb = wpool.tile([1, VS], F16, tag="bvoc")
                ones1v = wpool.tile([1, 128], F16, tag="ones1v")
                nc.vector.memset(ones1v[:], 1.0)

            cc_in = dram.tile([H * F, N], F16, name="cc_in")
            cc_out = dram.tile(
                [NCORES, H * F, N], F16, addr_space="Shared", name="cc_out"
            )
            sum_in = [
                dram.tile([128, GB[g + 1] - GB[g]], F32, name=f"sum_in{g}")
                for g in range(NG)
            ]
            sum_out = [
                dram.tile(
                    [128, GB[g + 1] - GB[g]], F32, addr_space="Shared", name=f"sum_out{g}"
                )
                for g in range(NG)
            ]

            # ==== phase A: embedding LN + GAT (own batch) ==============
            with (
                tc.tile_pool(name="pa", bufs=1) as pa,
                tc.tile_pool(name="pa_tmp", bufs=2) as pa_tmp,
                tc.tile_pool(name="ps_a", bufs=1, space="PSUM") as ps_a,
            ):
                # input DMAs in latency order: LN input first, the big
                # vocab weights (not needed until phase B) last.
                xp_sb = [pa.tile([128, D], F32, tag=f"xp{m}", name=f"xp{m}") for m in range(2)]
                for m in range(2):
                    nc.sync.dma_start(
                        out=xp_sb[m][:], in_=xpre[m * 128 : (m + 1) * 128, :]
                    )
                mneg_sb = [pa.tile([128, N], F16, tag=f"mneg{j}", name=f"mneg{j}") for j in range(2)]
                for jt in range(2):
                    nc.sync.dma_start(
                        out=mneg_sb[jt][:], in_=mneg[jt * 128 : (jt + 1) * 128, :]
                    )
                wmat_sb = [
                    pa.tile([128, H * F], F16, tag=f"wmat{kt}", name=f"wmat{kt}")
                    for kt in range(NKT)
                ]
                wsum_sb = [
                    pa.tile([128, 2 * H], F16, tag=f"wsum{kt}", name=f"wsum{kt}")
                    for kt in range(NKT)
                ]
                for kt in range(NKT):
                    nc.sync.dma_start(
                        out=wmat_sb[kt][:], in_=wmat[kt * 128 : (kt + 1) * 128, :]
                    )
                    nc.sync.dma_start(
                        out=wsum_sb[kt][:], in_=wsum[kt * 128 : (kt + 1) * 128, :]
                    )
                if with_ln_b:
                    browm_sb = pa.tile([1, H * F], F16, tag="browm")
                    nc.sync.dma_start(out=browm_sb[:], in_=browm[:, :])
                    brows_sb = pa.tile([1, 2 * H], F16, tag="brows")
                    nc.sync.dma_start(out=brows_sb[:], in_=brows[:, :])
                for kt in range(4):
                    nc.sync.dma_start(out=w_sb[kt][:], in_=wst[kt, :, :])
                if with_out_b:
                    nc.sync.dma_start(out=bvoc_sb[:], in_=bvoc[:, :])

                idw = pa.tile([128, 128], F16, tag="idw")
                bass_masks_identity(nc, idw[:])
                ones1 = pa.tile([1, 128], F16, tag="ones1")
                nc.vector.memset(ones1[:], 1.0)
                eps_sb = pa.tile([128, 1], F32, tag="eps_sb")
                nc.vector.memset(eps_sb[:], LN_EPS)

                # ---- LayerNorm (tokens on partitions) -----------------
                # mean via vector reduce in parallel with sum-of-squares via
                # scalar Square+accum; then var = E[x^2]-mu^2 on tiny tiles.
                xn_sb = [pa.tile([128, D], F16, tag=f"xn{m}", name=f"xn{m}") for m in range(2)]
                for m in range(2):
                    xp = xp_sb[m]
                    xsum = pa_tmp.tile([128, 1], F32, tag="xsum")
                    nc.vector.tensor_reduce(
                        out=xsum[:], in_=xp[:], axis=AX.X, op=OP.add
                    )
                    sq = pa_tmp.tile([128, D], F32, tag="sq")
                    ssum = pa_tmp.tile([128, 1], F32, tag="ssum")
                    nc.scalar.activation(
                        sq[:], xp[:], AF.Square, accum_out=ssum[:, 0:1]
                    )
                    mu = pa_tmp.tile([128, 1], F32, tag="mu")
                    nc.vector.tensor_scalar_mul(mu[:], xsum[:], 1.0 / D)
                    ex2 = pa_tmp.tile([128, 1], F32, tag="ex2")
                    nc.vector.tensor_scalar_mul(ex2[:], ssum[:], 1.0 / D)
                    mu2 = pa_tmp.tile([128, 1], F32, tag="mu2")
                    nc.vector.tensor_scalar_mul(mu2[:], mu[:], mu[:, 0:1])
                    var = pa_tmp.tile([128, 1], F32, tag="var")
                    nc.vector.scalar_tensor_tensor(
                        var[:], ex2[:], 0.0, mu2[:], OP.add, OP.subtract
                    )
                    sd = pa_tmp.tile([128, 1], F32, tag="sd")
                    nc.scalar.activation(
                        sd[:], var[:], AF.Sqrt, bias=eps_sb[:, 0:1]
                    )
                    rstd = pa_tmp.tile([128, 1], F32, tag="rstd")
                    nc.vector.reciprocal(rstd[:], sd[:])
                    nc.vector.tensor_scalar(
                        xn_sb[m][:], xp[:], mu[:, 0:1], rstd[:, 0:1],
                        OP.subtract, OP.mult,
                    )

                # ---- transpose xn -> xT[kt] [128 hid, 256 tok] --------
                xt_sb = [pa.tile([128, N], F16, tag=f"xt{kt}", name=f"xt{kt}") for kt in range(NKT)]
                for kt in range(NKT):
                    for m in range(2):
                        ptr = ps_a.tile([128, 128], F16, tag="ptr", bufs=2)
                        nc.tensor.transpose(
                            ptr[:], xn_sb[m][:, kt * 128 : (kt + 1) * 128], idw[:]
                        )
                        nc.vector.tensor_scalar_mul(
                            xt_sb[kt][:, m * 128 : (m + 1) * 128], ptr[:], 1.0
                        )

                # ---- all-heads Wh GEMM + s1/s2 contractions -----------
                wh_all = [pa.tile([128, H * F], F16, tag=f"whall{m}", name=f"whall{m}") for m in range(2)]
                s12m = [pa.tile([128, 2 * H], F32, tag=f"s12m{m}", name=f"s12m{m}") for m in range(2)]
                s1r = [pa.tile([1, N], F16, tag=f"s1r{h}", name=f"s1r{h}") for h in range(H)]
                for m in range(2):
                    pwh = ps_a.tile([128, H * F], F32, tag="pwh", bufs=2)
                    for kt in range(NKT):
                        nc.tensor.matmul(
                            pwh[:],
                            xt_sb[kt][:, m * 128 : (m + 1) * 128],
                            wmat_sb[kt][:],
                            start=(kt == 0),
                            stop=(kt == NKT - 1) and not with_ln_b,
                        )
                    if with_ln_b:
                        nc.tensor.matmul(
                            pwh[:], ones1[:], browm_sb[:], start=False, stop=True
                        )
                    nc.vector.tensor_scalar_mul(wh_all[m][:], pwh[:], 1.0)

                    pws = ps_a.tile([128, 2 * H], F32, tag="pws", bufs=1)
                    for kt in range(NKT):
                        nc.tensor.matmul(
                            pws[:],
                            xt_sb[kt][:, m * 128 : (m + 1) * 128],
                            wsum_sb[kt][:],
                            start=(kt == 0),
                            stop=(kt == NKT - 1) and not with_ln_b,
                        )
                    if with_ln_b:
                        nc.tensor.matmul(
                            pws[:], ones1[:], brows_sb[:], start=False, stop=True
                        )
                    nc.vector.tensor_scalar_mul(s12m[m][:], pws[:], 1.0)
                    # per-head s1 column [128,1] -> row [1,128] via transpose
                    s12h = pa_tmp.tile([128, H], F16, tag="s12h")
                    nc.vector.tensor_scalar_mul(s12h[:], pws[:, 0:H], 1.0)
                    for h in range(H):
                        ps1h = ps_a.tile([1, 128], F16, tag="ps1h", bufs=1)
                        nc.tensor.transpose(ps1h[:], s12h[:, h : h + 1], idw[:])
                        nc.vector.tensor_scalar_mul(s1r[h][:, m * 128 : (m + 1) * 128], ps1h[:], 1.0)

                # ---- per-head attention + aggregation -----------------
                att = [
                    [pa.tile([128, N], F16, tag=f"att{h}_{m}", name=f"att{h}_{m}") for m in range(2)]
                    for h in range(H)
                ]
                cat_sb = [pa.tile([128, N], F16, tag=f"cat{h}", name=f"cat{h}") for h in range(H)]

                for h in range(H):
                    # attention scores + column softmax (over i = free dim)
                    for jt in range(2):
                        # psum = broadcast(s1) + (-5000)*mask
                        pet = ps_a.tile([128, N], F32, tag="pet", bufs=2)
                        nc.tensor.matmul(
                            pet[:], ones1[:], s1r[h][:], start=True, stop=False
                        )
                        nc.tensor.matmul(
                            pet[:], idw[:], mneg_sb[jt][:], start=False, stop=True
                        )
                        et = pa_tmp.tile([128, N], F32, tag="et")
                        nc.vector.tensor_scalar_add(
                            et[:], pet[:], s12m[jt][:, H + h : H + h + 1]
                        )
                        lr = pa_tmp.tile([128, N], F32, tag="lr")
                        nc.vector.scalar_tensor_tensor(
                            lr[:], et[:], ALPHA, et[:], OP.mult, OP.max
                        )
                        # softmax without max-subtraction (f32 exp; max ~e19)
                        ex = pa_tmp.tile([128, N], F32, tag="ex")
                        asum = pa_tmp.tile([128, 1], F32, tag="asum")
                        nc.scalar.activation(
                            ex[:], lr[:], AF.Exp, accum_out=asum[:, 0:1]
                        )
                        rec = pa_tmp.tile([128, 1], F32, tag="rec")
                        nc.vector.reciprocal(rec[:], asum[:])
                        nc.vector.tensor_scalar_mul(
                            att[h][jt][:], ex[:], rec[:, 0:1]
                        )

                    # hp^T = Wh^T @ att^T, then elu -> catT rows of head h
                    php = ps_a.tile([128, N], F32, tag="pwh", bufs=2)
                    for jt in range(2):
                        nc.tensor.matmul(
                            php[:],
                            wh_all[jt][:, h * F : (h + 1) * F],
                            att[h][jt][:],
                            start=(jt == 0),
                            stop=(jt == 1),
                        )
                    e0h = pa_tmp.tile([128, N], F16, tag="e0h")
                    nc.scalar.activation(e0h[:], php[:], AF.Exp)
                    tmh = pa_tmp.tile([128, N], F16, tag="tmh")
                    nc.vector.tensor_scalar(
                        tmh[:], e0h[:], 1.0, -1.0, OP.min, OP.add
                    )
                    nc.vector.scalar_tensor_tensor(
                        cat_sb[h][:], php[:], 0.0, tmh[:], OP.max, OP.add
                    )
                    nc.sync.dma_start(
                        out=cc_in[h * F : (h + 1) * F, :],
                        in_=cat_sb[h][:],
                    )
                nc.gpsimd.collective_compute(
                    "AllGather",
                    OP.bypass,
                    replica_groups=rg,
                    ins=[cc_in.opt()],
                    outs=[cc_out.opt()],
                )
                for kt in range(4):
                    nc.sync.dma_start(
                        out=catf[kt][:].rearrange("p (r n) -> p r n", r=NCORES),
                        in_=cc_out[:, kt * F : (kt + 1) * F, :].rearrange(
                            "r p n -> p r n"
                        ),
                    )

            # ==== vocab-parallel output linear + log_softmax ===========
            with (
                tc.tile_pool(name="vp_pool", bufs=1) as vp_pool,
                tc.tile_pool(name="big_tmp", bufs=3) as big_tmp,
                tc.tile_pool(name="stat", bufs=1) as stat,
                tc.tile_pool(name="stage_pool", bufs=4) as stage_pool,
                tc.tile_pool(name="ps_z", bufs=8, space="PSUM") as ps_z,
            ):
                # q[m] holds elu(z) for m-tile m (f16)
                qt = [
                    vp_pool.tile([128, VS], F16, tag=f"q{m}", name=f"q{m}") for m in range(NM)
                ]
                lsq = stat.tile([128, NM * 2], F32, tag="lsq")
                gsum = stat.tile([128, NM], F32, tag="gsum")
                logl = stat.tile([128, NM], F32, tag="logl")
                nlogl = stat.tile([128, NM], F32, tag="nlogl")

                pend_dum = []

                def emit_dum(m, qi):
                    qc0, qcw = DCHUNKS[qi]
                    dum = big_tmp.tile([128, DW], F16, tag="dum", bufs=2)
                    nc.scalar.activation(
                        dum[:, 0:qcw],
                        qt[m][:, qc0 : qc0 + qcw],
                        AF.Exp,
                        accum_out=lsq[:, m * 2 + qi : m * 2 + qi + 1],
                    )

                def do_mtile(m):
                    """GEMM + elu chunks; dum halves interleaved.

                    The second dum half is deferred into the NEXT m-tile's
                    chunk stream (after its first exp1) so the scalar queue
                    never delays PSUM recycling at an m-tile boundary.
                    """
                    for ci, (c0, cw) in enumerate(CHUNKS):
                        zp = ps_z.tile([128, CW], F32, tag="z")
                        for kt in range(4):
                            nc.tensor.matmul(
                                zp[:, 0:cw],
                                catf[kt][:, m * 128 : (m + 1) * 128],
                                w_sb[kt][:, c0 : c0 + cw],
                                start=(kt == 0),
                                stop=(kt == 3) and not with_out_b,
                            )
                        if with_out_b:
                            nc.tensor.matmul(
                                zp[:, 0:cw],
                                ones1v[:],
                                bvoc_sb[:, c0 : c0 + cw],
                                start=False,
                                stop=True,
                            )
                        # e0 = exp(z) (f16; overflow->inf is clamped by min)
                        e0 = big_tmp.tile([128, CW], F16, tag="e0")
                        nc.scalar.activation(e0[:, 0:cw], zp[:, 0:cw], AF.Exp)
                        # tmin1 = min(e0,1) - 1
                        tm = big_tmp.tile([128, CW], F16, tag="tm")
                        nc.vector.tensor_scalar(
                            tm[:, 0:cw], e0[:, 0:cw], 1.0, -1.0, OP.min, OP.add
                        )
                        # q = max(z,0) + tmin1  == elu(z)
                        nc.vector.scalar_tensor_tensor(
                            qt[m][:, c0 : c0 + cw],
                            zp[:, 0:cw],
                            0.0,
                            tm[:, 0:cw],
                            OP.max,
                            OP.add,
                        )
                        # spread the exp-sum work across the chunk stream
                        if ci == 1 and pend_dum:
                            emit_dum(*pend_dum.pop(0))
                        elif ci == 4:
                            emit_dum(m, 0)
                    pend_dum.append((m, 1))

                def start_group_ar(g):
                    """Reduce group sums and fire its AllReduce.

                    The tiny sum DMAs ride the gpsimd DMA queue so they are
                    not stuck behind megabytes of output DMA on sync's queue.
                    """
                    g0, g1 = GB[g], GB[g + 1]
                    gm = g1 - g0
                    while pend_dum:
                        emit_dum(*pend_dum.pop(0))
                    ls = stat.tile([128, 8], F32, tag="ls", name=f"ls{g}")
                    nc.vector.tensor_reduce(
                        out=ls[:, 0:gm],
                        in_=lsq[:, g0 * 2 : g1 * 2].rearrange(
                            "p (m q) -> p m q", q=2
                        ),
                        axis=AX.X,
                        op=OP.add,
                    )
                    nc.gpsimd.dma_start(out=sum_in[g][:], in_=ls[:, 0:gm])
                    nc.gpsimd.collective_compute(
                        "AllReduce",
                        OP.add,
                        replica_groups=rg,
                        ins=[sum_in[g].opt()],
                        outs=[sum_out[g].opt()],
                    )
                def compute_logl(g):
                    """ln of the reduced sums, on the vector engine
                    (exponent bit-extract + degree-5 polynomial), so the
                    scalar activation table never leaves the exp set."""
                    g0, g1 = GB[g], GB[g + 1]
                    gm = g1 - g0
                    nc.gpsimd.dma_start(out=gsum[:, g0:g1], in_=sum_out[g][:])
                    # pad columns contribute exp(0)=1 each: subtract first
                    gs = gsum[:, g0:g1]
                    s6 = stat.tile([128, 8], F32, tag="s6", name=f"s6{g}")
                    nc.vector.tensor_scalar_add(s6[:, 0:gm], gs, -NPADC)
                    ui = s6[:, 0:gm].bitcast(mybir.dt.uint32)
                    ei = stat.tile([128, 8], mybir.dt.uint32, tag="ei", name=f"ei{g}")
                    nc.vector.tensor_scalar(
                        ei[:, 0:gm], ui, 23, None, OP.logical_shift_right
                    )
                    ef = stat.tile([128, 8], F32, tag="ef", name=f"ef{g}")
                    nc.vector.tensor_scalar_mul(ef[:, 0:gm], ei[:, 0:gm], 1.0)
                    mi = stat.tile([128, 8], mybir.dt.uint32, tag="mi", name=f"mi{g}")
                    nc.vector.tensor_scalar(
                        mi[:, 0:gm], ui, 0x007FFFFF, 0x3F800000,
                        OP.bitwise_and, OP.bitwise_or,
                    )
                    tv = stat.tile([128, 8], F32, tag="tv", name=f"tv{g}")
                    nc.vector.tensor_scalar_sub(
                        tv[:, 0:gm], mi[:, 0:gm].bitcast(F32), 1.0
                    )
                    pz = stat.tile([128, 8], F32, tag="pz", name=f"pz{g}")
                    nc.vector.tensor_scalar_mul(pz[:, 0:gm], tv[:, 0:gm], LNC[4])
                    for a in (LNC[3], LNC[2], LNC[1], LNC[0]):
                        nc.vector.scalar_tensor_tensor(
                            pz[:, 0:gm], pz[:, 0:gm], a, tv[:, 0:gm],
                            OP.add, OP.mult,
                        )
                    # exponent field is still biased by 127: fold into poly
                    nc.vector.tensor_scalar_add(
                        pz[:, 0:gm], pz[:, 0:gm], -127.0 * LN2
                    )
                    nc.vector.scalar_tensor_tensor(
                        logl[:, g0:g1], ef[:, 0:gm], LN2, pz[:, 0:gm],
                        OP.mult, OP.add,
                    )
                    nc.vector.tensor_scalar_mul(nlogl[:, g0:g1], logl[:, g0:g1], -1.0)

                def emit_final_mtile(m, last_group=False):
                    """out = q - log(L), staged f16: 3/4 vector, 1/4 scalar."""
                    for oi, (c0, cw) in enumerate(QCHUNKS):
                        stg = stage_pool.tile([128, QW], F16, tag="stg")
                        on_scalar = False
                        if not on_scalar:
                            nc.vector.tensor_scalar_sub(
                                stg[:, 0:cw],
                                qt[m][:, c0 : c0 + cw],
                                logl[:, m : m + 1],
                            )
                        else:
                            nc.scalar.activation(
                                stg[:, 0:cw],
                                qt[m][:, c0 : c0 + cw],
                                AF.Identity,
                                bias=nlogl[:, m : m + 1],
                            )
                        nc.sync.dma_start(
                            out=out[m * 128 : (m + 1) * 128, c0 : c0 + cw],
                            in_=stg[:, 0:cw],
                        )

                # staggered schedule: group g's AllReduce fires right after
                # its compute; its ln chain + finals are emitted a group
                # later (AR long done -> no queue stall), with the final
                # stages interleaved between m-tiles so they never form a
                # block that delays the next group's elementwise work.
                for g in range(NG):
                    for m in range(GB[g], GB[g + 1]):
                        do_mtile(m)
                    start_group_ar(g)
                    if g >= 1:
                        compute_logl(g - 1)
                        for m in range(GB[g - 1], GB[g]):
                            emit_final_mtile(m)
                compute_logl(NG - 1)
                for m in range(GB[NG - 1], GB[NG]):
                    emit_final_mtile(m, last_group=True)

    nc.compile()
    return nc


def bass_masks_identity(nc, ident_ap):
    from concourse import masks

    masks.make_identity(nc, ident_ap)


def _host_prep(inputs):
    """Per-core input maps from full inputs (numpy only)."""
    tok = np.asarray(inputs["token_ids"])
    typ = np.asarray(inputs["type_ids"])
    syn = np.asarray(inputs["synset_ids"])
    hw = np.asarray(inputs["highway"]).astype(bool)
    tok_emb = np.asarray(inputs["tok_emb"], dtype=np.float32)
    type_emb = np.asarray(inputs["type_emb"], dtype=np.float32)
    pos_emb = np.asarray(inputs["pos_emb"], dtype=np.float32)
    ln_g = np.asarray(inputs["ln_g"], dtype=np.float32)
    ln_b = np.asarray(inputs["ln_b"], dtype=np.float32)
    W = np.asarray(inputs["W"], dtype=np.float32)
    a = np.asarray(inputs["a"], dtype=np.float32)
    out_W = np.asarray(inputs["out_W"], dtype=np.float32)
    out_b = np.asarray(inputs["out_b"], dtype=np.float32)

    # embeddings (host gather + add, f32 like the reference)
    x_pre = tok_emb[tok] + type_emb[typ] + pos_emb[:N][None]  # (B,N,D)

    # graph mask (host index logic), transposed to [j, i]
    vis = syn[:, :, None] == syn[:, None, :]
    s1m = (typ == 1) & hw
    s3m = (typ == 3) & hw
    d1 = np.isin(typ, [0, 2, 5]) & hw
    d3 = np.isin(typ, [6, 4, 0]) & hw
    vis = vis | (s1m[:, :, None] & d1[:, None, :]) | (s3m[:, :, None] & d3[:, None, :])
    mask = vis & (tok != 0)[:, None, :]  # (B,N,N) over [i,j]
    # -5000 where masked-out, 0 where visible; [j, i] layout
    mneg = np.where(mask.transpose(0, 2, 1), 0.0, MASK_NEG).astype(np.float16)

    # GAT weights: fold ln_g; separate Wh matrix and a1/a2 contractions
    Wg = W * ln_g[None, :, None]  # (H,D,F)
    a1, a2 = a[:, :F], a[:, F:]
    c1 = np.einsum("hdf,hf->hd", Wg, a1)  # (H,D)
    c2 = np.einsum("hdf,hf->hd", Wg, a2)
    wmat = Wg.transpose(1, 0, 2).reshape(D, H * F).astype(np.float16)
    wsum = np.concatenate([c1.T, c2.T], axis=1).astype(np.float16)  # (D, 2H)

    with_ln_b = bool(np.any(ln_b != 0.0))
    browm = brows = None
    if with_ln_b:
        b1 = np.einsum("hdf,hf->hd", W, a1)  # (H,D)
        b2 = np.einsum("hdf,hf->hd", W, a2)
        browm = np.einsum("d,hdf->hf", ln_b, W).reshape(1, H * F).astype(np.float16)
        brows = np.concatenate([b1 @ ln_b, b2 @ ln_b]).reshape(1, 2 * H).astype(
            np.float16
        )

    # vocab shards of out_W^T (padded to 30528)
    wpad = np.zeros((VPAD, H * F), dtype=np.float32)
    wpad[:V] = out_W
    with_out_b = bool(np.any(out_b != 0.0))
    bpad = np.zeros((VPAD,), dtype=np.float32)
    bpad[:V] = out_b

    in_maps = []
    for c in range(NCORES):
        wc = wpad[c * VS : (c + 1) * VS].T.astype(np.float16)  # (512, VS)
        m = {
            "xpre": np.ascontiguousarray(x_pre[c]),
            "mneg": np.ascontiguousarray(mneg[c]),
            "wmat": wmat,
            "wsum": wsum,
            "wst": np.ascontiguousarray(wc.reshape(4, 128, VS)),
        }
        if with_ln_b:
            m["browm"] = browm
            m["brows"] = brows
        if with_out_b:
            m["bvoc"] = np.ascontiguousarray(
                bpad[c * VS : (c + 1) * VS].reshape(1, VS).astype(np.float16)
            )
        in_maps.append(m)
    return in_maps, with_ln_b, with_out_b


def kernel(**inputs) -> np.ndarray:
    in_maps, with_ln_b, with_out_b = _host_prep(inputs)

    key = (with_ln_b, with_out_b)
    if key not in _NC_CACHE:
        _NC_CACHE[key] = _build(with_ln_b, with_out_b)
    nc = _NC_CACHE[key]

    trace = bool(int(os.environ.get("KBERT_TRACE", "0")))
    res = run_bass_kernel_spmd(
        nc, in_maps, core_ids=list(range(NCORES)), trace=trace
    )
    if trace and res.exec_time_ns is not None:
        print(f"HW exec time: {res.exec_time_ns} ns")
        if res.instructions_and_trace is not None:
            print(f"trace: {res.instructions_and_trace[1]}")

    full = np.empty((B * N, VPAD), dtype=np.float32)
    for c in range(NCORES):
        full[:, c * VS : (c + 1) * VS] = res.results[c]["out"]
    return np.ascontiguousarray(full[:, :V].reshape(B, N, V))
